# revision 23
# baseline (speedup 1.0000x reference)
"""Causal self-attention (B=4, T=2048, C=1024, H=16, D=64) on 8 trn2 cores.

Sharding: data-parallel over B (4) x tensor-parallel over head-halves (2).
Core c handles batch c//2 with heads [8*(c%2), 8*(c%2)+8). Each core emits a
partial projection output [2048, 1024] (bf16); host sums the two head-half
partials per batch and adds the (bv @ Wp + bp) correction row.

Device layout highlights:
 - matmul cost on PE is proportional to the output free-dim size only, so the
   attention*V product is computed in [t, d] layout: per (head, t-block of
   128, s-tile) one bf16 matmul with a 65-wide output (64 V columns + a ones
   column that yields the softmax denominator Z per t-partition). That makes
   softmax normalization a per-partition reciprocal+tensor_scalar (no
   broadcast matmuls) and halves the AV stream cost vs. the [d, t] form.
 - normalized [t, d-pair] tiles are transposed back to [d-pair, t] on the PE
   (bf16 is_transpose matmul vs. a 128x128 identity) to feed the output
   projection; the transpose emission is delayed by one head so the DVE
   normalize chain never stalls the PE.
 - S^T = K^T.T @ Q^T stays fp32r in [s, t] layout in [128,1024] psum pair
   slots (one exp instruction per slot amortizes ACT's fixed per-instruction
   overhead); all S matmuls are >= 256 wide for the 1 cyc/row fp32r rate.
 - the attention phase is ACT(exp)-bound, so emission interleaves S pair
   units with the previous head's AV chains and with projection / QKV
   chains at fine granularity; attention chunks run in order 1,2,3,0 so the
   pipeline drains on the cheapest chunk's exps.
 - weights/x/y stream as bf16 in 1-2 large strided DMAs each: the HWDGE
   descriptor generator is a serial ~630ns/DMA resource, so few big
   transfers beat many tile-sized ones.
 - softmax skips max-subtraction (logits are ~N(0,1); exp cannot overflow)
 - causal masking via 0/1 mask multiply on the 4 diagonal-block patterns
"""

import os
import sys

for _p in ("/opt/trn_rl_repo", "/root/.axon_site/_ro/trn_rl_repo"):
    if os.path.isdir(_p) and _p not in sys.path:
        sys.path.insert(0, _p)

import numpy as np
from concourse import bacc, masks, mybir, tile
from concourse.bass_utils import run_bass_kernel_spmd

N_CORES = 8
B, T, C = 4, 2048, 1024
H, D = 16, 64          # full model heads
HG = 8                 # heads per core (head-group)
CH = HG * D            # 512, per-core qkv width
NT = T // 128          # 16 s-tiles
NJ = T // 512          # 4 t-chunks
NC_ = C // 128         # 8 contraction tiles
F32 = mybir.dt.float32
F32R = mybir.dt.float32r
BF16 = mybir.dt.bfloat16
U16 = mybir.dt.uint16
AF = mybir.ActivationFunctionType

ET_BUFS = 18           # [128,1024] bf16 S^T pair tiles across the head pipeline

_CACHE = {}


def _emit(nc, tc, aps):
    xT, wq, wk, wv, wp, bq2, bk2, mask, yout = (
        aps["xT"], aps["wq"], aps["wk"], aps["wv"], aps["wp"],
        aps["bq2"], aps["bk2"], aps["mask"], aps["y"],
    )

    pool = tc.alloc_tile_pool(name="pool", bufs=1)
    psp = tc.alloc_tile_pool(name="ps", bufs=1, space="PSUM")

    # ---- persistent tensors ----
    kt = [pool.tile([128, T], F32R, name=f"kt{m}", tag="kt", bufs=4) for m in range(4)]
    vp = [pool.tile([128, 520], BF16, name=f"vp{i}", tag="vp", bufs=NT)
          for i in range(NT)]
    # single lower-triangle mask (1{s <= t}) for the diagonal 128x128 blocks
    tri_f = pool.tile([128, 128], F32, name="tri_f", tag="tri_f", bufs=1)
    tri = pool.tile([128, 128], BF16, name="tri", tag="tri", bufs=1)
    ident = pool.tile([128, 128], BF16, name="ident", tag="ident", bufs=1)
    bqs = pool.tile([128, 4], F32, name="bqs", tag="bias", bufs=2)
    bks = pool.tile([128, 4], F32, name="bks", tag="bias", bufs=2)
    ones_b = pool.tile([128, 8], BF16, name="ones_b", tag="ones_b", bufs=1)

    # qkv weights: one [128, 8x512] tile per matrix, loaded in 1-2 big DMAs
    # (the HWDGE descriptor generator is serial at ~630ns/DMA)
    wqb = pool.tile([128, 4096], BF16, name="wqb", tag="wqkv", bufs=3)
    wkb = pool.tile([128, 4096], BF16, name="wkb", tag="wqkv", bufs=3)
    wvb = pool.tile([128, 4096], BF16, name="wvb", tag="wqkv", bufs=3)
    wqs = [wqb[:, 512 * ci:512 * ci + 512] for ci in range(NC_)]
    wks = [wkb[:, 512 * ci:512 * ci + 512] for ci in range(NC_)]
    wvs = [wvb[:, 512 * ci:512 * ci + 512] for ci in range(NC_)]
    wpb = pool.tile([128, 4096], BF16, name="wpb", tag="wp", bufs=1)
    wps = [[wpb[:, 1024 * m + 512 * n:1024 * m + 512 * n + 512] for n in range(2)]
           for m in range(4)]

    def _w3d(ap):  # [128, 4096] tile -> [128, 8, 512] view
        return ap.rearrange("p (ci c) -> p ci c", c=512)

    # startup DMAs: every large transfer serializes through the shared HWDGE
    # generator + DMA engines, so they all go on the sync queue in strict
    # consumption order (q operands, then k, v, next x chunk); the scalar
    # queue only carries the tiny bias/mask loads.
    xtb = [pool.tile([128, 4096], BF16, name=f"xtb{j}", tag="xt", bufs=2)
           for j in range(NJ)]
    xts_all = [[xtb[j][:, 512 * ci:512 * ci + 512] for ci in range(NC_)]
               for j in range(NJ)]

    def _x3d(j, lo, hi):
        return (xtb[j][:].rearrange("p (ci c) -> p ci c", c=512)[:, lo:hi],
                xT[128 * lo:128 * hi, 512 * j:512 * j + 512]
                .rearrange("(ci p) c -> p ci c", p=128).bitcast(BF16))

    def _wq3d(lo, hi):
        return (_w3d(wqb[:])[:, lo:hi],
                wq[128 * lo:128 * hi, :]
                .rearrange("(ci p) c -> p ci c", p=128).bitcast(BF16))

    nc.sync.dma_start(*_wq3d(0, 1))
    nc.scalar.dma_start(bqs[:], bq2[:])
    nc.scalar.dma_start(bks[:], bk2[:])
    nc.sync.dma_start(*_x3d(0, 0, 1))
    nc.sync.dma_start(*_wq3d(1, 4))
    nc.sync.dma_start(*_x3d(0, 1, 4))
    nc.sync.dma_start(*_wq3d(4, 8))
    nc.sync.dma_start(*_x3d(0, 4, 8))
    nc.sync.dma_start(_w3d(wkb[:]),
                      wk[:, :].rearrange("(ci p) c -> p ci c", p=128).bitcast(BF16))
    nc.sync.dma_start(_w3d(wvb[:]),
                      wv[:, :].rearrange("(ci p) c -> p ci c", p=128).bitcast(BF16))
    nc.scalar.dma_start(tri_f[:], mask[:])
    nc.vector.tensor_copy(tri[:], tri_f[:])
    nc.gpsimd.memset(ones_b[:], 1.0)
    masks.make_identity(nc, ident[:])

    qtc = [[None] * NJ for _ in range(4)]   # per-chunk Q^T tiles
    otc = [[None] * NJ for _ in range(4)]   # per-chunk O^T tiles
    nrmt = [[None] * 4 for _ in range(4)]   # per-pair normalized [t, d-pair]

    def emit_qkv_dma(j):
        if j == 0:
            return
        nc.sync.dma_start(xtb[j][:].rearrange("p (ci c) -> p ci c", c=512),
                          xT[:, 512 * j:512 * j + 512]
                          .rearrange("(ci p) c -> p ci c", p=128).bitcast(BF16))

    def qkv_chain(j, kind, m):
        # one [128,512] psum accumulation chain of the q/k/v projections
        xts = xts_all[j]
        if kind == "v":
            i = 4 * j + m
            ps = psp.tile([128, 512], F32, name=f"vps{i}", tag="qk", bufs=2)
            for ci in range(NC_):
                nc.tensor.matmul(
                    ps[:], xts[ci][:, 128 * m:128 * m + 128], wvs[ci],
                    start=(ci == 0), stop=(ci == NC_ - 1),
                )
            dst = vp[i][:, 0:520].rearrange("p (h e) -> p h e", e=65)[:, :, 0:64]
            src = ps[:].rearrange("p (h e) -> p h e", e=64)
            nc.vector.tensor_copy(dst, src)
            ocol = vp[i][:, 0:520].rearrange("p (h e) -> p h e", e=65)[:, :, 64:65]
            nc.vector.tensor_copy(ocol, ones_b[:].unsqueeze(2))
            return
        wsrc, bias_t = (wqs, bqs) if kind == "q" else (wks, bks)
        ps = psp.tile([128, 512], F32, name=f"{kind}ps{j}_{m}", tag="qk", bufs=2)
        for ci in range(NC_):
            nc.tensor.matmul(
                ps[:], wsrc[ci][:, 128 * m:128 * m + 128], xts[ci][:],
                start=(ci == 0), stop=(ci == NC_ - 1),
            )
        if kind == "k":
            out_ap = kt[m][:, 512 * j:512 * j + 512]
        else:
            t_ = pool.tile([128, 512], F32R, name=f"qt{m}_{j}", tag="qtc", bufs=16)
            qtc[m][j] = t_
            out_ap = t_[:]
        nc.vector.tensor_scalar_add(out_ap, ps[:], bias_t[:, m:m + 1])

    def qkv_units(j):
        # q first (unblocks attention), then k, then v
        return ([lambda j=j, m=m: qkv_chain(j, "q", m) for m in range(4)]
                + [lambda j=j, m=m: qkv_chain(j, "k", m) for m in range(4)]
                + [lambda j=j, m=m: qkv_chain(j, "v", m) for m in range(4)])

    def emit_qkv(j):
        emit_qkv_dma(j)
        for f in qkv_units(j):
            f()

    # ---- attention ----
    # S^T pair-slot descriptors for chunk j: list of slots, each a list of
    # (i, col0, t0, w, mask_col). AV consumption: col = col0 + 128*u - t0.
    def s_slots(j):
        out = []
        for p in range(2 * j):  # full pairs
            out.append([(2 * p, 0, 0, 512, None), (2 * p + 1, 512, 0, 512, None)])
        # diagonal pair A: tiles 4j (full width) and 4j+1 (t >= 128)
        out.append([(4 * j, 0, 0, 512, 0), (4 * j + 1, 512, 128, 384, 512)])
        # diagonal pair B: tiles 4j+2 and 4j+3, both 256 wide at t0=256
        # (tile 4j+3 only needs t in [384,512) but a 256-wide matmul avoids
        # the <256 fp32r 4x penalty; its cols [256,384) are acausal garbage
        # that the exp covers harmlessly and AV never reads)
        out.append([(4 * j + 2, 0, 256, 256, 0), (4 * j + 3, 256, 256, 256, 384)])
        return out

    av_pend = []  # cross-call head pipeline: AV trails S by one head
    tp_pend = []  # transposes trail their pair's AV by one head

    def s_units(j, h):
        mt, off = h // 2, 64 * (h % 2)
        ets = {}  # s-tile i -> (et tile, col0, t0)

        def emit_slot(slot):
            qsrc = qtc[mt][j]
            wtot = max(c0 + w for (_, c0, _, w, _) in slot)
            sp = psp.tile([128, 1024], F32, name=f"sp{h}_{j}", tag="sp", bufs=2)
            et = pool.tile([128, 1024], BF16, name=f"et{h}_{j}", tag="et",
                           bufs=ET_BUFS)
            for (i, c0, t0, w, _) in slot:
                nc.tensor.matmul(
                    sp[:, c0:c0 + w], kt[mt][off:off + 64, 128 * i:128 * i + 128],
                    qsrc[off:off + 64, t0:t0 + w], start=True, stop=True,
                )
            nc.scalar.activation(et[:, 0:wtot], sp[:, 0:wtot], AF.Exp, scale=0.125)
            for (i, c0, t0, w, mcol) in slot:
                if mcol is not None:
                    nc.vector.tensor_mul(et[:, mcol:mcol + 128],
                                         et[:, mcol:mcol + 128], tri[:])
                ets[i] = (et, c0, t0)

        units = [lambda slot=slot: emit_slot(slot) for slot in s_slots(j)]
        return units, ets

    def av_units(j, h, ets):
        mt, off = h // 2, 64 * (h % 2)
        op = psp.tile([128, 260], F32, name=f"o{h}_{j}", tag="o", bufs=2)

        def chain(u):
            oc = 65 * u
            lo = 4 * j + u + 1
            for i in range(lo):
                et, c0, t0 = ets[i]
                col = c0 + 128 * u - t0
                nc.tensor.matmul(
                    op[:, oc:oc + 65], et[:, col:col + 128],
                    vp[i][:, 65 * h:65 * h + 65],
                    start=(i == 0), stop=(i == lo - 1),
                )
            # normalize in [t, d]: Z is column 64 -> per-partition scalar
            rb = pool.tile([128, 1], F32, name=f"rb{h}_{j}_{u}", tag="rb", bufs=16)
            nc.vector.reciprocal(rb[:], op[:, oc + 64:oc + 65])
            if h % 2 == 0:
                nrmt[mt][u] = pool.tile([128, 128], BF16, name=f"nrm{mt}_{j}_{u}",
                                        tag="nrm", bufs=24)
            nc.vector.tensor_scalar_mul(
                nrmt[mt][u][:, off:off + 64], op[:, oc:oc + 64], rb[:]
            )

        def tp_unit(mt=mt, j=j, pair_nrm=nrmt[mt]):
            # pair complete: transpose [t, d-pair] -> [d-pair, t] for the proj
            tp = psp.tile([128, 512], BF16, name=f"tp{mt}_{j}", tag="o", bufs=2)
            ot = pool.tile([128, 512], BF16, name=f"ot{mt}_{j}", tag="otc", bufs=16)
            otc[mt][j] = ot
            for u in range(4):
                nc.tensor.matmul(
                    tp[:, 128 * u:128 * u + 128], pair_nrm[u][:], ident[:],
                    is_transpose=True,
                )
                nc.vector.tensor_copy(
                    ot[:, 128 * u:128 * u + 128], tp[:, 128 * u:128 * u + 128]
                )

        units = [lambda u=u: chain(u) for u in range(4)]
        return units, (tp_unit if h % 2 == 1 else None)

    def attn_seq(plan):
        """plan: list of ((j, h), [extra units]) or ((j, h), extras, "hold").
        Emit S for each head, interleaving the previous head's AV chains, the
        previous pair's transposes and the extra PE work units (proj/QKV
        chains). A "hold" entry skips draining the AV queue at that head,
        deepening the S->AV pipeline for the heads that follow."""
        for entry in plan:
            (j, h), head_extras = entry[0], entry[1]
            hold = len(entry) > 2
            su, ets = s_units(j, h)
            # transposes delayed from an earlier head run first; the one
            # produced by this head's av_units must wait until the next head
            tpu_now = tp_pend.pop(0) if tp_pend else None
            au = []
            if av_pend and not hold:
                au, tpu = av_units(*av_pend.pop(0))
                if tpu is not None:
                    tp_pend.append(tpu)
            seq = [su[0]]
            if len(su) > 1:
                seq.append(su[1])
            if tpu_now is not None:
                seq.append(tpu_now)
            si, ai = 2, 0
            while si < len(su) or ai < len(au):
                if ai < len(au):
                    seq.append(au[ai])
                    ai += 1
                if si < len(su):
                    seq.append(su[si])
                    si += 1
            for f in seq:
                f()
            for f in head_extras:
                f()
            av_pend.append((j, h, ets))

    def flush_av():
        while av_pend:
            au, tpu = av_units(*av_pend.pop(0))
            for f in au:
                f()
            if tpu is not None:
                tp_pend.append(tpu)
        while tp_pend:
            tp_pend.pop(0)()

    def emit_wp_loads():
        nc.sync.dma_start(wpb[:].rearrange("p (m c) -> p m c", c=1024),
                          wp[:, :].rearrange("(m p) c -> p m c", p=128).bitcast(BF16))

    def proj_unit(j, u):
        # both column halves of one [128 t, 1024] output row block; the DMA
        # is split per half so the first half streams out under the second
        t = 4 * j + u
        yo = pool.tile([128, 1024], BF16, name=f"yo{t}", tag="yo", bufs=4)
        for n in range(2):
            ps = psp.tile([128, 512], F32, name=f"yps{t}_{n}", tag="qk", bufs=2)
            for m in range(4):
                nc.tensor.matmul(
                    ps[:], otc[m][j][:, 128 * u:128 * u + 128], wps[m][n],
                    start=(m == 0), stop=(m == 3),
                )
            nc.vector.tensor_copy(yo[:, 512 * n:512 * n + 512], ps[:])
            nc.sync.dma_start(
                yout[128 * t:128 * t + 128, 512 * n:512 * n + 512].bitcast(BF16),
                yo[:, 512 * n:512 * n + 512],
            )

    def proj_units(j):
        return [lambda j=j, u=u: proj_unit(j, u) for u in range(4)]

    # ---- macro schedule ----
    # attn(1) runs against qkv(2) chains as PE filler; then the heads of
    # chunks 2, 3 and 0 are interleaved so the exp-heavy chunk-3 heads
    # alternate with cheap chunk-0 heads, with qkv(3)/proj chains placed at
    # the remaining ACT-bound positions. The pipeline drains on chunk-0 exps
    # and proj(3)/proj(0) end the program as pure PE+DMA work.
    emit_qkv(0)
    emit_qkv(1)
    emit_qkv_dma(2)
    qk2, qk3 = qkv_units(2), qkv_units(3)
    p1, p2, p3 = proj_units(1), proj_units(2), proj_units(3)
    attn_seq([((1, h), ex) for h, ex in enumerate((
        qk2[0:2], qk2[2:4], qk2[4:6], qk2[6:8],
        qk2[8:9], qk2[9:10], qk2[10:11], qk2[11:12]))])
    emit_qkv_dma(3)
    emit_wp_loads()
    p0 = proj_units(0)
    attn_seq([
        ((2, 0), qk3[0:2]),    # q01
        ((2, 1), qk3[2:4]),    # q23
        ((2, 2), qk3[4:6]),    # k01
        ((2, 3), qk3[6:10]),   # k23 v01
        ((3, 0), qk3[10:12]),  # v23
        ((0, 0), []),
        ((2, 4), [p1[0]]),
        ((0, 1), []),
        ((3, 1), [p1[1]]),
        ((0, 2), []),
        ((2, 5), []),
        ((0, 3), []),
        ((3, 2), [p1[3]]),
        ((2, 6), [p1[2]]),
        ((3, 3), []),
        ((2, 7), []),
        ((3, 4), []),
        ((3, 5), p2[0:2]),
        ((3, 6), p2[2:4]),
        ((3, 7), []),
        ((0, 4), [], "hold"),
        ((0, 5), []),
        ((0, 6), p3[0:2]),
        ((0, 7), p3[2:4]),
    ])
    flush_av()
    for f in p0:
        f()

    for m in range(4):
        qtc[m] = [None] * NJ
        otc[m] = [None] * NJ
    pool.release()
    psp.release()


def build(passes=1):
    key = ("nc", passes)
    if key in _CACHE:
        return _CACHE[key]
    nc = bacc.Bacc("TRN2", target_bir_lowering=False, debug=False,
                   num_devices=N_CORES)
    aps = {
        "xT": nc.dram_tensor("xT", [C, T], U16, kind="ExternalInput").ap(),
        "wq": nc.dram_tensor("wq", [C, CH], U16, kind="ExternalInput").ap(),
        "wk": nc.dram_tensor("wk", [C, CH], U16, kind="ExternalInput").ap(),
        "wv": nc.dram_tensor("wv", [C, CH], U16, kind="ExternalInput").ap(),
        "wp": nc.dram_tensor("wp", [CH, C], U16, kind="ExternalInput").ap(),
        "bq2": nc.dram_tensor("bq2", [128, 4], F32, kind="ExternalInput").ap(),
        "bk2": nc.dram_tensor("bk2", [128, 4], F32, kind="ExternalInput").ap(),
        "mask": nc.dram_tensor("mask", [128, 128], F32, kind="ExternalInput").ap(),
        "y": nc.dram_tensor("y", [T, C], U16, kind="ExternalOutput").ap(),
    }
    with tile.TileContext(nc) as tc:
        for _ in range(passes):
            _emit(nc, tc, aps)
    nc.compile()
    _CACHE[key] = nc
    return nc


def _bf16_bits(a):
    """float32 ndarray -> bfloat16 bit pattern as uint16 (round to nearest even)."""
    u = np.ascontiguousarray(a, dtype=np.float32).view(np.uint32)
    r = (u + 0x7FFF + ((u >> 16) & 1)) >> 16
    return r.astype(np.uint16)


def _bf16_to_f32(bits):
    return (bits.astype(np.uint32) << 16).view(np.float32)


def make_in_maps(x, Wq, bq, Wk, bk, Wv, bv, Wp, bp):
    # lower-triangle 0/1 mask for the diagonal 128x128 attention blocks
    s_idx = np.arange(128)[:, None]
    t_idx = np.arange(128)[None, :]
    mask = (s_idx <= t_idx).astype(np.float32)
    in_maps = []
    for c in range(N_CORES):
        b, g = c // 2, c % 2
        cols = slice(CH * g, CH * g + CH)
        in_maps.append({
            "xT": _bf16_bits(x[b].T),
            "wq": _bf16_bits(Wq[:, cols]),
            "wk": _bf16_bits(Wk[:, cols]),
            "wv": _bf16_bits(Wv[:, cols]),
            "wp": _bf16_bits(Wp[cols, :]),
            "bq2": np.ascontiguousarray(bq[cols].reshape(4, 128).T),
            "bk2": np.ascontiguousarray(bk[cols].reshape(4, 128).T),
            "mask": mask,
        })
    return in_maps


def kernel(x, Wq, bq, Wk, bk, Wv, bv, Wp, bp):
    # host-side prep is pure numpy; convert in case jax arrays are passed
    x, Wq, bq, Wk, bk, Wv, bv, Wp, bp = (
        np.asarray(a, dtype=np.float32)
        for a in (x, Wq, bq, Wk, bk, Wv, bv, Wp, bp)
    )
    nc = build()
    in_maps = make_in_maps(x, Wq, bq, Wk, bk, Wv, bv, Wp, bp)
    # the axon-proxied device occasionally reports a transient unrecoverable
    # exec state that clears on a fresh attempt; retry rather than fail
    last_err = None
    for _attempt in range(3):
        try:
            res = run_bass_kernel_spmd(nc, in_maps, core_ids=list(range(N_CORES)))
            break
        except Exception as e:  # noqa: BLE001
            last_err = e
            import time as _time
            _time.sleep(5)
    else:
        raise last_err
    corr = (bv @ Wp + bp).astype(np.float32)
    out = np.empty((B, T, C), dtype=np.float32)
    for b in range(B):
        out[b] = (_bf16_to_f32(res.results[2 * b]["y"])
                  + _bf16_to_f32(res.results[2 * b + 1]["y"]) + corr)
    return out


# revision 24
# speedup vs baseline: 1.0396x; 1.0396x over previous
"""Causal self-attention (B=4, T=2048, C=1024, H=16, D=64) on 8 trn2 cores.

Sharding: data-parallel over B (4) x tensor-parallel over head-halves (2).
Core c handles batch c//2 with heads [8*(c%2), 8*(c%2)+8). Each core emits a
partial projection output [2048, 1024] (bf16); host sums the two head-half
partials per batch and adds the (bv @ Wp + bp) correction row.

Device layout highlights:
 - matmul cost on PE is proportional to the output free-dim size only, so the
   attention*V product is computed in [t, d] layout: per (head, t-block of
   128, s-tile) one bf16 matmul with a 65-wide output (64 V columns + a ones
   column that yields the softmax denominator Z per t-partition). That makes
   softmax normalization a per-partition reciprocal+tensor_scalar (no
   broadcast matmuls) and halves the AV stream cost vs. the [d, t] form.
 - normalized [t, d-pair] tiles are transposed back to [d-pair, t] on the PE
   (bf16 is_transpose matmul vs. a 128x128 identity) to feed the output
   projection; the transpose emission is delayed by one head so the DVE
   normalize chain never stalls the PE.
 - S^T = K^T.T @ Q^T stays fp32r in [s, t] layout in [128,1024] psum pair
   slots (one exp instruction per slot amortizes ACT's fixed per-instruction
   overhead); all S matmuls are >= 256 wide for the 1 cyc/row fp32r rate.
 - the attention phase is ACT(exp)-bound, so emission interleaves S pair
   units with the previous head's AV chains and with projection / QKV
   chains at fine granularity; attention chunks run in order 1,2,3,0 so the
   pipeline drains on the cheapest chunk's exps.
 - weights/x/y stream as bf16 in 1-2 large strided DMAs each: the HWDGE
   descriptor generator is a serial ~630ns/DMA resource, so few big
   transfers beat many tile-sized ones.
 - softmax skips max-subtraction (logits are ~N(0,1); exp cannot overflow)
 - causal masking via 0/1 mask multiply on the 4 diagonal-block patterns
"""

import os
import sys

for _p in ("/opt/trn_rl_repo", "/root/.axon_site/_ro/trn_rl_repo"):
    if os.path.isdir(_p) and _p not in sys.path:
        sys.path.insert(0, _p)

import numpy as np
from concourse import bacc, masks, mybir, tile
from concourse.bass_utils import run_bass_kernel_spmd

N_CORES = 8
B, T, C = 4, 2048, 1024
H, D = 16, 64          # full model heads
HG = 8                 # heads per core (head-group)
CH = HG * D            # 512, per-core qkv width
NT = T // 128          # 16 s-tiles
NJ = T // 512          # 4 t-chunks
NC_ = C // 128         # 8 contraction tiles
F32 = mybir.dt.float32
F32R = mybir.dt.float32r
BF16 = mybir.dt.bfloat16
U16 = mybir.dt.uint16
AF = mybir.ActivationFunctionType

ET_BUFS = 18           # [128,1024] bf16 S^T pair tiles across the head pipeline

_CACHE = {}


def _emit(nc, tc, aps):
    xT, wq, wk, wv, wp, bq2, bk2, mask, yout = (
        aps["xT"], aps["wq"], aps["wk"], aps["wv"], aps["wp"],
        aps["bq2"], aps["bk2"], aps["mask"], aps["y"],
    )

    pool = tc.alloc_tile_pool(name="pool", bufs=1)
    psp = tc.alloc_tile_pool(name="ps", bufs=1, space="PSUM")

    # ---- persistent tensors ----
    kt = [pool.tile([128, T], F32R, name=f"kt{m}", tag="kt", bufs=4) for m in range(4)]
    vp = [pool.tile([128, 520], BF16, name=f"vp{i}", tag="vp", bufs=NT)
          for i in range(NT)]
    # single lower-triangle mask (1{s <= t}) for the diagonal 128x128 blocks
    tri_f = pool.tile([128, 128], F32, name="tri_f", tag="tri_f", bufs=1)
    tri = pool.tile([128, 128], BF16, name="tri", tag="tri", bufs=1)
    ident = pool.tile([128, 128], BF16, name="ident", tag="ident", bufs=1)
    bqs = pool.tile([128, 4], F32, name="bqs", tag="bias", bufs=2)
    bks = pool.tile([128, 4], F32, name="bks", tag="bias", bufs=2)
    ones_b = pool.tile([128, 8], BF16, name="ones_b", tag="ones_b", bufs=1)

    # qkv weights: one [128, 8x512] tile per matrix, loaded in 1-2 big DMAs
    # (the HWDGE descriptor generator is serial at ~630ns/DMA)
    wqb = pool.tile([128, 4096], BF16, name="wqb", tag="wqkv", bufs=3)
    wkb = pool.tile([128, 4096], BF16, name="wkb", tag="wqkv", bufs=3)
    wvb = pool.tile([128, 4096], BF16, name="wvb", tag="wqkv", bufs=3)
    wqs = [wqb[:, 512 * ci:512 * ci + 512] for ci in range(NC_)]
    wks = [wkb[:, 512 * ci:512 * ci + 512] for ci in range(NC_)]
    wvs = [wvb[:, 512 * ci:512 * ci + 512] for ci in range(NC_)]
    wpb = pool.tile([128, 4096], BF16, name="wpb", tag="wp", bufs=1)
    wps = [[wpb[:, 1024 * m + 512 * n:1024 * m + 512 * n + 512] for n in range(2)]
           for m in range(4)]

    def _w3d(ap):  # [128, 4096] tile -> [128, 8, 512] view
        return ap.rearrange("p (ci c) -> p ci c", c=512)

    # startup DMAs: every large transfer serializes through the shared HWDGE
    # generator + DMA engines, so they all go on the sync queue in strict
    # consumption order (q operands, then k, v, next x chunk); the scalar
    # queue only carries the tiny bias/mask loads.
    xtb = [pool.tile([128, 4096], BF16, name=f"xtb{j}", tag="xt", bufs=2)
           for j in range(NJ)]
    xts_all = [[xtb[j][:, 512 * ci:512 * ci + 512] for ci in range(NC_)]
               for j in range(NJ)]

    def _x3d(j, lo, hi):
        return (xtb[j][:].rearrange("p (ci c) -> p ci c", c=512)[:, lo:hi],
                xT[128 * lo:128 * hi, 512 * j:512 * j + 512]
                .rearrange("(ci p) c -> p ci c", p=128).bitcast(BF16))

    def _wq3d(lo, hi):
        return (_w3d(wqb[:])[:, lo:hi],
                wq[128 * lo:128 * hi, :]
                .rearrange("(ci p) c -> p ci c", p=128).bitcast(BF16))

    nc.sync.dma_start(*_wq3d(0, 1))
    nc.scalar.dma_start(bqs[:], bq2[:])
    nc.scalar.dma_start(bks[:], bk2[:])
    nc.sync.dma_start(*_x3d(0, 0, 1))
    nc.sync.dma_start(*_wq3d(1, 4))
    nc.sync.dma_start(*_x3d(0, 1, 4))
    nc.sync.dma_start(*_wq3d(4, 8))
    nc.sync.dma_start(*_x3d(0, 4, 8))
    nc.sync.dma_start(_w3d(wkb[:]),
                      wk[:, :].rearrange("(ci p) c -> p ci c", p=128).bitcast(BF16))
    nc.sync.dma_start(_w3d(wvb[:]),
                      wv[:, :].rearrange("(ci p) c -> p ci c", p=128).bitcast(BF16))
    nc.scalar.dma_start(tri_f[:], mask[:])
    nc.vector.tensor_copy(tri[:], tri_f[:])
    nc.gpsimd.memset(ones_b[:], 1.0)
    masks.make_identity(nc, ident[:])

    qtc = [[None] * NJ for _ in range(4)]   # per-chunk Q^T tiles
    otc = [[None] * NJ for _ in range(4)]   # per-chunk O^T tiles
    nrmt = [[None] * 4 for _ in range(4)]   # per-pair normalized [t, d-pair]

    def emit_qkv_dma(j):
        if j == 0:
            return
        nc.sync.dma_start(xtb[j][:].rearrange("p (ci c) -> p ci c", c=512),
                          xT[:, 512 * j:512 * j + 512]
                          .rearrange("(ci p) c -> p ci c", p=128).bitcast(BF16))

    def qkv_chain(j, kind, m):
        # one [128,512] psum accumulation chain of the q/k/v projections
        xts = xts_all[j]
        if kind == "v":
            i = 4 * j + m
            ps = psp.tile([128, 512], F32, name=f"vps{i}", tag="qk", bufs=2)
            for ci in range(NC_):
                nc.tensor.matmul(
                    ps[:], xts[ci][:, 128 * m:128 * m + 128], wvs[ci],
                    start=(ci == 0), stop=(ci == NC_ - 1),
                )
            dst = vp[i][:, 0:520].rearrange("p (h e) -> p h e", e=65)[:, :, 0:64]
            src = ps[:].rearrange("p (h e) -> p h e", e=64)
            nc.vector.tensor_copy(dst, src)
            ocol = vp[i][:, 0:520].rearrange("p (h e) -> p h e", e=65)[:, :, 64:65]
            nc.vector.tensor_copy(ocol, ones_b[:].unsqueeze(2))
            return
        wsrc, bias_t = (wqs, bqs) if kind == "q" else (wks, bks)
        ps = psp.tile([128, 512], F32, name=f"{kind}ps{j}_{m}", tag="qk", bufs=2)
        for ci in range(NC_):
            nc.tensor.matmul(
                ps[:], wsrc[ci][:, 128 * m:128 * m + 128], xts[ci][:],
                start=(ci == 0), stop=(ci == NC_ - 1),
            )
        if kind == "k":
            out_ap = kt[m][:, 512 * j:512 * j + 512]
        else:
            t_ = pool.tile([128, 512], F32R, name=f"qt{m}_{j}", tag="qtc", bufs=16)
            qtc[m][j] = t_
            out_ap = t_[:]
        nc.vector.tensor_scalar_add(out_ap, ps[:], bias_t[:, m:m + 1])

    def qkv_units(j):
        # q first (unblocks attention), then k, then v
        return ([lambda j=j, m=m: qkv_chain(j, "q", m) for m in range(4)]
                + [lambda j=j, m=m: qkv_chain(j, "k", m) for m in range(4)]
                + [lambda j=j, m=m: qkv_chain(j, "v", m) for m in range(4)])

    def emit_qkv(j):
        emit_qkv_dma(j)
        for f in qkv_units(j):
            f()

    # ---- attention ----
    # S^T pair-slot descriptors for chunk j: list of slots, each a list of
    # (i, col0, t0, w, mask_col). AV consumption: col = col0 + 128*u - t0.
    def s_slots(j):
        out = []
        for p in range(2 * j):  # full pairs
            out.append([(2 * p, 0, 0, 512, None), (2 * p + 1, 512, 0, 512, None)])
        # diagonal pair A: tiles 4j (full width) and 4j+1 (t >= 128)
        out.append([(4 * j, 0, 0, 512, 0), (4 * j + 1, 512, 128, 384, 512)])
        # diagonal pair B: tiles 4j+2 and 4j+3, both 256 wide at t0=256
        # (tile 4j+3 only needs t in [384,512) but a 256-wide matmul avoids
        # the <256 fp32r 4x penalty; its cols [256,384) are acausal garbage
        # that the exp covers harmlessly and AV never reads)
        out.append([(4 * j + 2, 0, 256, 256, 0), (4 * j + 3, 256, 256, 256, 384)])
        return out

    av_pend = []  # cross-call head pipeline: AV trails S by one head
    tp_pend = []  # transposes trail their pair's AV by one head

    def s_units(j, h):
        mt, off = h // 2, 64 * (h % 2)
        ets = {}  # s-tile i -> (et tile, col0, t0)

        def emit_slot(slot):
            qsrc = qtc[mt][j]
            wtot = max(c0 + w for (_, c0, _, w, _) in slot)
            sp = psp.tile([128, 1024], F32, name=f"sp{h}_{j}", tag="sp", bufs=2)
            et = pool.tile([128, 1024], BF16, name=f"et{h}_{j}", tag="et",
                           bufs=ET_BUFS)
            for (i, c0, t0, w, _) in slot:
                nc.tensor.matmul(
                    sp[:, c0:c0 + w], kt[mt][off:off + 64, 128 * i:128 * i + 128],
                    qsrc[off:off + 64, t0:t0 + w], start=True, stop=True,
                )
            nc.scalar.activation(et[:, 0:wtot], sp[:, 0:wtot], AF.Exp, scale=0.125)
            for (i, c0, t0, w, mcol) in slot:
                if mcol is not None:
                    nc.vector.tensor_mul(et[:, mcol:mcol + 128],
                                         et[:, mcol:mcol + 128], tri[:])
                ets[i] = (et, c0, t0)

        units = [lambda slot=slot: emit_slot(slot) for slot in s_slots(j)]
        return units, ets

    def av_units(j, h, ets):
        mt, off = h // 2, 64 * (h % 2)
        op = psp.tile([128, 260], F32, name=f"o{h}_{j}", tag="o", bufs=2)

        def chain(u):
            oc = 65 * u
            lo = 4 * j + u + 1
            for i in range(lo):
                et, c0, t0 = ets[i]
                col = c0 + 128 * u - t0
                nc.tensor.matmul(
                    op[:, oc:oc + 65], et[:, col:col + 128],
                    vp[i][:, 65 * h:65 * h + 65],
                    start=(i == 0), stop=(i == lo - 1),
                )
            # normalize in [t, d]: Z is column 64 -> per-partition scalar
            rb = pool.tile([128, 1], F32, name=f"rb{h}_{j}_{u}", tag="rb", bufs=16)
            nc.vector.reciprocal(rb[:], op[:, oc + 64:oc + 65])
            if h % 2 == 0:
                nrmt[mt][u] = pool.tile([128, 128], BF16, name=f"nrm{mt}_{j}_{u}",
                                        tag="nrm", bufs=24)
            nc.vector.tensor_scalar_mul(
                nrmt[mt][u][:, off:off + 64], op[:, oc:oc + 64], rb[:]
            )

        def tp_unit(mt=mt, j=j, pair_nrm=nrmt[mt]):
            # pair complete: transpose [t, d-pair] -> [d-pair, t] for the proj
            tp = psp.tile([128, 512], BF16, name=f"tp{mt}_{j}", tag="o", bufs=2)
            ot = pool.tile([128, 512], BF16, name=f"ot{mt}_{j}", tag="otc", bufs=16)
            otc[mt][j] = ot
            for u in range(4):
                nc.tensor.matmul(
                    tp[:, 128 * u:128 * u + 128], pair_nrm[u][:], ident[:],
                    is_transpose=True,
                )
                nc.vector.tensor_copy(
                    ot[:, 128 * u:128 * u + 128], tp[:, 128 * u:128 * u + 128]
                )

        units = [lambda u=u: chain(u) for u in range(4)]
        return units, (tp_unit if h % 2 == 1 else None)

    def attn_seq(plan):
        """plan: list of ((j, h), [extra units]) or ((j, h), extras, "hold").
        Emit S for each head, interleaving the previous head's AV chains, the
        previous pair's transposes and the extra PE work units (proj/QKV
        chains). A "hold" entry skips draining the AV queue at that head,
        deepening the S->AV pipeline for the heads that follow."""
        for entry in plan:
            (j, h), head_extras = entry[0], entry[1]
            hold = len(entry) > 2
            su, ets = s_units(j, h)
            # transposes delayed from an earlier head run first; the one
            # produced by this head's av_units must wait until the next head
            tpu_now = tp_pend.pop(0) if tp_pend else None
            au = []
            if av_pend and not hold:
                au, tpu = av_units(*av_pend.pop(0))
                if tpu is not None:
                    tp_pend.append(tpu)
            seq = [su[0]]
            if len(su) > 1:
                seq.append(su[1])
            if tpu_now is not None:
                seq.append(tpu_now)
            si, ai = 2, 0
            while si < len(su) or ai < len(au):
                if ai < len(au):
                    seq.append(au[ai])
                    ai += 1
                if si < len(su):
                    seq.append(su[si])
                    si += 1
            for f in seq:
                f()
            for f in head_extras:
                f()
            av_pend.append((j, h, ets))

    def flush_av():
        while av_pend:
            au, tpu = av_units(*av_pend.pop(0))
            for f in au:
                f()
            if tpu is not None:
                tp_pend.append(tpu)
        while tp_pend:
            tp_pend.pop(0)()

    def emit_wp_loads():
        nc.sync.dma_start(wpb[:].rearrange("p (m c) -> p m c", c=1024),
                          wp[:, :].rearrange("(m p) c -> p m c", p=128).bitcast(BF16))

    def proj_unit(j, u):
        # both column halves of one [128 t, 1024] output row block; the DMA
        # is split per half so the first half streams out under the second
        t = 4 * j + u
        yo = pool.tile([128, 1024], BF16, name=f"yo{t}", tag="yo", bufs=4)
        for n in range(2):
            ps = psp.tile([128, 512], F32, name=f"yps{t}_{n}", tag="qk", bufs=2)
            for m in range(4):
                nc.tensor.matmul(
                    ps[:], otc[m][j][:, 128 * u:128 * u + 128], wps[m][n],
                    start=(m == 0), stop=(m == 3),
                )
            nc.vector.tensor_copy(yo[:, 512 * n:512 * n + 512], ps[:])
            nc.sync.dma_start(
                yout[128 * t:128 * t + 128, 512 * n:512 * n + 512].bitcast(BF16),
                yo[:, 512 * n:512 * n + 512],
            )

    def proj_units(j):
        return [lambda j=j, u=u: proj_unit(j, u) for u in range(4)]

    # ---- macro schedule ----
    # attn(1) runs against qkv(2) chains as PE filler; then the heads of
    # chunks 2, 3 and 0 are interleaved so the exp-heavy chunk-3 heads
    # alternate with cheap chunk-0 heads, with qkv(3)/proj chains placed at
    # the remaining ACT-bound positions. The pipeline drains on chunk-0 exps
    # and proj(3)/proj(0) end the program as pure PE+DMA work.
    emit_qkv(0)
    emit_qkv(1)
    emit_qkv_dma(2)
    qk2, qk3 = qkv_units(2), qkv_units(3)
    p1, p2, p3 = proj_units(1), proj_units(2), proj_units(3)
    attn_seq([((1, h), ex) for h, ex in enumerate((
        qk2[0:2], qk2[2:4], qk2[4:6], qk2[6:8],
        qk2[8:9], qk2[9:10], qk2[10:11], qk2[11:12]))])
    emit_qkv_dma(3)
    emit_wp_loads()
    p0 = proj_units(0)
    attn_seq([
        ((2, 0), qk3[0:2]),    # q01
        ((2, 1), qk3[2:4]),    # q23
        ((2, 2), qk3[4:6]),    # k01
        ((2, 3), qk3[6:10]),   # k23 v01
        ((3, 0), qk3[10:12]),  # v23
        ((0, 0), []),
        ((2, 4), [p1[0]]),
        ((0, 1), []),
        ((3, 1), [p1[1]]),
        ((0, 2), []),
        ((2, 5), [p1[2]]),
        ((0, 3), []),
        ((3, 2), [p1[3]]),
        ((0, 4), []),
        ((2, 6), []),
        ((0, 5), []),
        ((3, 3), []),
        ((0, 6), []),
        ((2, 7), []),
        ((0, 7), []),
        ((3, 4), p2[0:2]),
        ((3, 5), p2[2:4]),
        ((3, 6), p0[0:2]),
        ((3, 7), p0[2:4]),
    ])
    flush_av()
    for f in p3:
        f()

    for m in range(4):
        qtc[m] = [None] * NJ
        otc[m] = [None] * NJ
    pool.release()
    psp.release()


def build(passes=1):
    key = ("nc", passes)
    if key in _CACHE:
        return _CACHE[key]
    nc = bacc.Bacc("TRN2", target_bir_lowering=False, debug=False,
                   num_devices=N_CORES)
    aps = {
        "xT": nc.dram_tensor("xT", [C, T], U16, kind="ExternalInput").ap(),
        "wq": nc.dram_tensor("wq", [C, CH], U16, kind="ExternalInput").ap(),
        "wk": nc.dram_tensor("wk", [C, CH], U16, kind="ExternalInput").ap(),
        "wv": nc.dram_tensor("wv", [C, CH], U16, kind="ExternalInput").ap(),
        "wp": nc.dram_tensor("wp", [CH, C], U16, kind="ExternalInput").ap(),
        "bq2": nc.dram_tensor("bq2", [128, 4], F32, kind="ExternalInput").ap(),
        "bk2": nc.dram_tensor("bk2", [128, 4], F32, kind="ExternalInput").ap(),
        "mask": nc.dram_tensor("mask", [128, 128], F32, kind="ExternalInput").ap(),
        "y": nc.dram_tensor("y", [T, C], U16, kind="ExternalOutput").ap(),
    }
    with tile.TileContext(nc) as tc:
        for _ in range(passes):
            _emit(nc, tc, aps)
    nc.compile()
    _CACHE[key] = nc
    return nc


def _bf16_bits(a):
    """float32 ndarray -> bfloat16 bit pattern as uint16 (round to nearest even)."""
    u = np.ascontiguousarray(a, dtype=np.float32).view(np.uint32)
    r = (u + 0x7FFF + ((u >> 16) & 1)) >> 16
    return r.astype(np.uint16)


def _bf16_to_f32(bits):
    return (bits.astype(np.uint32) << 16).view(np.float32)


def make_in_maps(x, Wq, bq, Wk, bk, Wv, bv, Wp, bp):
    # lower-triangle 0/1 mask for the diagonal 128x128 attention blocks
    s_idx = np.arange(128)[:, None]
    t_idx = np.arange(128)[None, :]
    mask = (s_idx <= t_idx).astype(np.float32)
    in_maps = []
    for c in range(N_CORES):
        b, g = c // 2, c % 2
        cols = slice(CH * g, CH * g + CH)
        in_maps.append({
            "xT": _bf16_bits(x[b].T),
            "wq": _bf16_bits(Wq[:, cols]),
            "wk": _bf16_bits(Wk[:, cols]),
            "wv": _bf16_bits(Wv[:, cols]),
            "wp": _bf16_bits(Wp[cols, :]),
            "bq2": np.ascontiguousarray(bq[cols].reshape(4, 128).T),
            "bk2": np.ascontiguousarray(bk[cols].reshape(4, 128).T),
            "mask": mask,
        })
    return in_maps


def kernel(x, Wq, bq, Wk, bk, Wv, bv, Wp, bp):
    # host-side prep is pure numpy; convert in case jax arrays are passed
    x, Wq, bq, Wk, bk, Wv, bv, Wp, bp = (
        np.asarray(a, dtype=np.float32)
        for a in (x, Wq, bq, Wk, bk, Wv, bv, Wp, bp)
    )
    nc = build()
    in_maps = make_in_maps(x, Wq, bq, Wk, bk, Wv, bv, Wp, bp)
    # the axon-proxied device occasionally reports a transient unrecoverable
    # exec state that clears on a fresh attempt; retry rather than fail
    last_err = None
    for _attempt in range(3):
        try:
            res = run_bass_kernel_spmd(nc, in_maps, core_ids=list(range(N_CORES)))
            break
        except Exception as e:  # noqa: BLE001
            last_err = e
            import time as _time
            _time.sleep(5)
    else:
        raise last_err
    corr = (bv @ Wp + bp).astype(np.float32)
    out = np.empty((B, T, C), dtype=np.float32)
    for b in range(B):
        out[b] = (_bf16_to_f32(res.results[2 * b]["y"])
                  + _bf16_to_f32(res.results[2 * b + 1]["y"]) + corr)
    return out
